# revision 37
# baseline (speedup 1.0000x reference)
"""Trainium2 Bass kernel for nn_Attention_56736517980223.

Full-input contract: kernel(**inputs) takes the unsharded inputs and returns
the full [2, 2048, 2048] attention output. Internally: tensor-parallel over
heads across 8 NeuronCores (1 KV head + 4 Q heads per core); each core
computes its heads' attention and a partial x@wo contribution; the host sums
the 8 partials.

Device-side dataflow per core (matmuls in bf16 with fp32 PSUM accumulate):
  - xT (host-pretransposed [dim, tok]) streams as the moving operand of the
    QKV projections; weight columns are even/odd-permuted so RoPE acts on
    contiguous 32-partition blocks.
  - scores are computed transposed (S^T[k, q] = kT.T-chunk @ qT) so softmax
    needs no reductions: exp(scale*s) runs on ScalarE straight out of PSUM,
    causal masking is a staircase zero-fill (gpsimd affine_select), and the
    row sums fall out of the PV matmul via a ones-column appended to V.
  - PV output [d+1, q] is normalized with reciprocal + partition_broadcast
    and written into attnT, which feeds the wo matmul as the stationary
    operand.
  - The wo projection is interleaved into the attention loop (one
    [128-token x 1024-col] unit per two kc iterations) so its matmuls fill
    the PE idle left by ScalarE's exp gating; output partials are written
    bf16 and summed on the host in fp32.
"""

import numpy as np

DIM = 2048
N_HEADS = 32
N_KV_HEADS = 8
HEAD_DIM = 64
BATCH = 2
SEQ = 2048
TOK = BATCH * SEQ  # 4096
N_CORES = 8
HPC = N_HEADS // N_KV_HEADS  # 4 q heads per core
CHUNK = 512  # token chunk (projection streaming / q block)
KC = 128     # key chunk (scores partition dim)
NQB = SEQ // CHUNK   # 4 q blocks per batch
NKC = SEQ // KC      # 16 key chunks per batch
SCALE = 1.0 / np.sqrt(HEAD_DIM)

_CACHE = {}
LAST_RESULT = None


def _build(tile_types, generic):
    """Build the SPMD Bass program.

    tile_types[qc][kc] in {'full', 'diag', 'skip', 'gen'} (batch-local,
    shared across batches and heads). 'diag' uses the causal affine_select;
    'gen' adds a DMA'd mask tile (only in generic mode).
    """
    from contextlib import ExitStack
    import concourse.bass as bass
    import concourse.tile as tile
    from concourse import bacc, mybir

    F32 = mybir.dt.float32
    F32R = mybir.dt.float32r
    BF16 = mybir.dt.bfloat16
    U16 = mybir.dt.uint16
    AF = mybir.ActivationFunctionType
    ALU = mybir.AluOpType

    nc = bacc.Bacc("TRN2", target_bir_lowering=False, debug=False,
                   num_devices=N_CORES)

    NCH = TOK // CHUNK  # 8 token chunks
    NKT = DIM // KC     # 16 contraction tiles for projections

    # All streaming inputs are host-packed so every DMA reads per-partition
    # CONTIGUOUS bytes (128 big descriptors instead of 2048 x 1KB gathers);
    # out is block-major per [128-token x 1024-col] wo unit for the same
    # reason, and the host reassembles.
    xt = nc.dram_tensor("xt", [NCH, KC, NKT * CHUNK], BF16,
                        kind="ExternalInput").ap()
    wq = nc.dram_tensor("wq", [KC, NKT * 2 * KC], BF16,
                        kind="ExternalInput").ap()
    wkv = nc.dram_tensor("wkv", [KC, NKT * KC], BF16,
                         kind="ExternalInput").ap()
    wo1 = nc.dram_tensor("wo1", [KC, DIM], BF16, kind="ExternalInput").ap()
    wo2 = nc.dram_tensor("wo2", [KC, DIM], BF16, kind="ExternalInput").ap()
    cos_q = nc.dram_tensor("cos_q", [NCH, KC, CHUNK], BF16,
                           kind="ExternalInput").ap()
    sin_q = nc.dram_tensor("sin_q", [NCH, KC, CHUNK], BF16,
                           kind="ExternalInput").ap()
    if generic:
        maskt = nc.dram_tensor("maskt", [SEQ, SEQ], F32,
                               kind="ExternalInput").ap()
    out = nc.dram_tensor("out", [TOK // KC, 2, KC, DIM // 2], BF16,
                         kind="ExternalOutput").ap()

    with tile.TileContext(nc) as tc, ExitStack() as ctx:
        persist = ctx.enter_context(tc.tile_pool(name="persist", bufs=1))
        qt1 = persist.tile([KC, TOK], BF16)   # heads 0,1 (rows 0:64 / 64:128)
        qt2 = persist.tile([KC, TOK], BF16)   # heads 2,3
        kt = persist.tile([KC, TOK], BF16)    # rows 0:64 = kT, 64:128 = dup
        ident = persist.tile([64, 64], BF16)
        nc.gpsimd.memset(ident[:].bitcast(U16), 0)
        nc.gpsimd.affine_select(
            out=ident[:], in_=ident[:], compare_op=ALU.not_equal,
            fill=1.0, base=0, channel_multiplier=1, pattern=[[-1, 64]])

        vpool = ctx.enter_context(tc.tile_pool(name="vpool", bufs=1))
        vt = vpool.tile([64, TOK], BF16)
        v_all = vpool.tile([KC, (TOK // KC) * 65], BF16)  # 32 [128,65] blocks

        # ---------------- projection + RoPE + V transpose ----------------
        with tc.tile_pool(name="proj", bufs=1) as proj, \
             tc.tile_pool(name="projs", bufs=3) as projs, \
             tc.tile_pool(name="ropet", bufs=2) as ropet, \
             tc.tile_pool(name="pps", bufs=2, space="PSUM") as pps:
            wq_sb = proj.tile([KC, NKT * 2 * KC], BF16)
            wkv_sb = proj.tile([KC, NKT * KC], BF16)
            nc.sync.dma_start(wq_sb[:], wq[:])
            nc.sync.dma_start(wkv_sb[:], wkv[:])

            def emit_transposes(tch):
                for j in range(CHUNK // KC):
                    blk = tch * (CHUNK // KC) + j
                    vp = pps.tile([KC, 64], BF16, tag="vp", name="vp",
                                  bufs=2)
                    nc.tensor.transpose(
                        vp[:], vt[0:64, blk * KC:(blk + 1) * KC], ident[:])
                    nc.scalar.copy(v_all[:, blk * 65:blk * 65 + 64], vp[:])
                    nc.gpsimd.memset(
                        v_all[:, blk * 65 + 64:blk * 65 + 65].bitcast(U16),
                        16256)

            for ch in range(NCH):
                tsl = slice(ch * CHUNK, (ch + 1) * CHUNK)
                # whole x chunk in one contiguous per-partition DMA
                xc = projs.tile([KC, NKT * CHUNK], BF16, tag="xc")
                nc.sync.dma_start(xc[:], xt[ch])
                cos_sb = projs.tile([KC, CHUNK], BF16)
                sin_sb = projs.tile([KC, CHUNK], BF16)
                nc.sync.dma_start(cos_sb[:], cos_q[ch])
                nc.sync.dma_start(sin_sb[:], sin_q[ch])

                a_ps = pps.tile([KC, CHUNK], F32, tag="a_ps")
                b_ps = pps.tile([KC, CHUNK], F32, tag="b_ps")
                kv_ps = pps.tile([KC, CHUNK], F32, tag="kv_ps")
                # group matmuls per accumulator: dense same-bank runs keep
                # the PE stream from cycling PSUM banks every instruction
                for ps_t, woff, wsb in ((a_ps, 0, wq_sb), (b_ps, 128, wq_sb),
                                        (kv_ps, 0, wkv_sb)):
                    stride = 256 if wsb is wq_sb else 128
                    for kti in range(NKT):
                        nc.tensor.matmul(
                            ps_t[:],
                            wsb[:, kti * stride + woff:
                                kti * stride + woff + 128],
                            xc[:, kti * CHUNK:(kti + 1) * CHUNK],
                            start=(kti == 0), stop=(kti == NKT - 1))

                # Q RoPE: A' = A*c - B*s ; B' = A*s + B*c. VectorE does the
                # four PSUM-reading multiplies full-width; GpSimd (idle here,
                # SBUF-only) combines 32-row slices straight into the
                # per-head-contiguous qt1/qt2 rows, keeping the qa/qb scatter
                # off the saturated DMA pipe.
                t1 = ropet.tile([KC, CHUNK], F32, tag="t1")
                t2 = ropet.tile([KC, CHUNK], F32, tag="t2")
                t3 = ropet.tile([KC, CHUNK], F32, tag="t3")
                t4 = ropet.tile([KC, CHUNK], F32, tag="t4")
                nc.vector.tensor_mul(t1[:], a_ps[:], cos_sb[:])
                nc.vector.tensor_mul(t2[:], b_ps[:], sin_sb[:])
                nc.vector.tensor_mul(t3[:], a_ps[:], sin_sb[:])
                nc.vector.tensor_mul(t4[:], b_ps[:], cos_sb[:])
                for h in range(HPC):
                    dst = qt1 if h < 2 else qt2
                    r0 = 64 * (h % 2)
                    hs = slice(32 * h, 32 * h + 32)
                    nc.gpsimd.tensor_sub(dst[r0:r0 + 32, tsl],
                                         t1[hs, :], t2[hs, :])
                    nc.gpsimd.tensor_add(dst[r0 + 32:r0 + 64, tsl],
                                         t3[hs, :], t4[hs, :])

                # K RoPE into kt rows 0:64 (kv_ps rows 0:32=x0, 32:64=x1)
                k1 = ropet.tile([32, CHUNK], F32, tag="k1")
                k2 = ropet.tile([32, CHUNK], F32, tag="k2")
                k3 = ropet.tile([32, CHUNK], F32, tag="k3")
                k4 = ropet.tile([32, CHUNK], F32, tag="k4")
                nc.vector.tensor_mul(k1[:], kv_ps[0:32, :], cos_sb[0:32, :])
                nc.vector.tensor_mul(k2[:], kv_ps[32:64, :], sin_sb[0:32, :])
                nc.vector.tensor_mul(k3[:], kv_ps[0:32, :], sin_sb[0:32, :])
                nc.vector.tensor_mul(k4[:], kv_ps[32:64, :], cos_sb[0:32, :])
                nc.gpsimd.tensor_sub(kt[0:32, tsl], k1[:], k2[:])
                nc.gpsimd.tensor_add(kt[32:64, tsl], k3[:], k4[:])
                # duplicate kT rows for base-64 rhs matmuls
                nc.gpsimd.tensor_copy(kt[64:128, tsl], kt[0:64, tsl])
                # V: evacuate psum rows 64:128 to vt on ScalarE (idle in this
                # phase); the transposes are deferred one chunk so the PE
                # stream never waits on this chunk's evacuation
                nc.scalar.copy(vt[0:64, tsl], kv_ps[64:128, :])
                if ch > 0:
                    emit_transposes(ch - 1)
            emit_transposes(NCH - 1)

        # ---------------- attention ----------------
        attn = ctx.enter_context(tc.tile_pool(name="attn", bufs=1))
        attnt1 = attn.tile([KC, TOK], BF16)
        attnt2 = attn.tile([KC, TOK], BF16)
        wo1_sb = attn.tile([KC, DIM], BF16)
        wo2_sb = attn.tile([KC, DIM], BF16)
        nc.sync.dma_start(wo1_sb[:], wo1[:])
        nc.sync.dma_start(wo2_sb[:], wo2[:])

        with tc.tile_pool(name="att", bufs=2) as att, \
             tc.tile_pool(name="atps", bufs=1, space="PSUM") as atps:
            # wo is interleaved into the attention loop: completed q-blocks
            # queue [128-token x 1024-col] output units that are emitted one
            # per two kc iterations, filling the PE idle left by ScalarE's
            # exp gating. PSUM: s(2x2) + o(2x1) + pv0/pv1(1x1 each) = 8 banks.
            pending = []
            nunit = [0]
            tick = [0]

            def emit_wo(unit, tag="o", bufs=1):
                m, nh = unit
                msl = slice(m * KC, (m + 1) * KC)
                o_ps = atps.tile([KC, 2 * CHUNK], F32, tag=tag,
                                 name="o_ps", bufs=bufs)
                for j in range(2):
                    nsl = slice(nh * 1024 + j * CHUNK,
                                nh * 1024 + (j + 1) * CHUNK)
                    psl = slice(j * CHUNK, (j + 1) * CHUNK)
                    nc.tensor.matmul(o_ps[:, psl], attnt1[:, msl],
                                     wo1_sb[:, nsl], start=True, stop=False)
                    nc.tensor.matmul(o_ps[:, psl], attnt2[:, msl],
                                     wo2_sb[:, nsl], start=False, stop=True)
                o_sb = att.tile([KC, 2 * CHUNK], BF16, tag="o_sb",
                                name="o_sb", bufs=4)
                nunit[0] += 1
                if nunit[0] % 2 == 0:
                    nc.vector.tensor_copy(o_sb[:], o_ps[:])
                else:
                    nc.scalar.copy(o_sb[:], o_ps[:])
                nc.sync.dma_start(out[m, nh], o_sb[:])

            def maybe_wo():
                tick[0] += 1
                if tick[0] % 2 == 0 and pending:
                    emit_wo(pending.pop(0))

            for b in range(BATCH):
                for qc in range(NQB):
                    q0 = qc * CHUNK                    # batch-local q offset
                    gq = slice(b * SEQ + q0, b * SEQ + q0 + CHUNK)
                    kcs = [k for k in range(NKC)
                           if tile_types[qc][k] != 'skip']
                    for pair in range(2):
                        qt = qt1 if pair == 0 else qt2
                        pv = [atps.tile([65, CHUNK], F32, tag=f"pv{hh}",
                                        name=f"pv{hh}", bufs=1)
                              for hh in range(2)]

                        def emit_pv(job):
                            ex_t, jw0, jst, jsp, jvblk = job
                            for hh in range(2):
                                csl = slice(hh * CHUNK + jw0,
                                            (hh + 1) * CHUNK)
                                nc.tensor.matmul(
                                    pv[hh][:, jw0:CHUNK],
                                    v_all[:, jvblk * 65:jvblk * 65 + 65],
                                    ex_t[:, csl], start=jst, stop=jsp)
                            maybe_wo()

                        prev_pv = None
                        for i, kci in enumerate(kcs):
                            k0 = kci * KC
                            gk = slice(b * SEQ + k0, b * SEQ + k0 + KC)
                            ty = tile_types[qc][kci]
                            st = (i == 0)
                            sp = (i == len(kcs) - 1)
                            # diag tiles only need q >= k0: shrink to cols
                            # [w0:CHUNK) (earlier cols are fully masked)
                            w0 = max(0, k0 - q0) if ty == 'diag' else 0
                            W = CHUNK - w0
                            gqw = slice(b * SEQ + q0 + w0,
                                        b * SEQ + q0 + CHUNK)
                            s_ps = atps.tile([KC, 2 * CHUNK], F32,
                                             tag="s", name="s_ps", bufs=2)
                            nc.tensor.matmul(
                                s_ps[:, w0:CHUNK], kt[0:64, gk],
                                qt[0:64, gqw], start=True, stop=True)
                            nc.tensor.matmul(
                                s_ps[:, CHUNK + w0:], kt[64:128, gk],
                                qt[64:128, gqw], start=True, stop=True)
                            ex = att.tile([KC, 2 * CHUNK], BF16,
                                          tag="ex", name="ex", bufs=3)
                            if ty == 'gen':
                                mt = att.tile([KC, CHUNK], F32, tag="mt",
                                              name="mt", bufs=4)
                                nc.sync.dma_start(
                                    mt[:], maskt[k0:k0 + KC, q0:q0 + CHUNK])
                                for hh in range(2):
                                    csl = slice(hh * CHUNK, (hh + 1) * CHUNK)
                                    tm = att.tile([KC, CHUNK], F32, tag="tm",
                                                  name="tm", bufs=4)
                                    nc.vector.scalar_tensor_tensor(
                                        tm[:], s_ps[:, csl], SCALE, mt[:],
                                        op0=ALU.mult, op1=ALU.add)
                                    nc.scalar.activation(
                                        ex[:, csl], tm[:], AF.Exp)
                            elif w0 == 0:
                                nc.scalar.activation(
                                    ex[:], s_ps[:], AF.Exp, scale=SCALE)
                            else:
                                sv = s_ps[:].rearrange(
                                    "p (h w) -> p h w", h=2)[:, :, w0:]
                                ev = ex[:].rearrange(
                                    "p (h w) -> p h w", h=2)[:, :, w0:]
                                nc.scalar.activation(ev, sv, AF.Exp,
                                                     scale=SCALE)
                            if ty == 'diag':
                                ev = ex[:].rearrange(
                                    "p (h w) -> p h w", h=2)[:, :, w0:]
                                nc.gpsimd.affine_select(
                                    out=ev, in_=ev,
                                    compare_op=ALU.is_ge, fill=0.0,
                                    base=q0 + w0 - k0,
                                    channel_multiplier=-1,
                                    pattern=[[0, 2], [1, W]])
                            vblk = (b * SEQ + k0) // KC
                            # pv is deferred one iteration: the PE issues the
                            # next tile's scores while ScalarE runs this exp,
                            # so it never stalls on the exp semaphore
                            if prev_pv is not None:
                                emit_pv(prev_pv)
                            prev_pv = (ex, w0, st, sp, vblk)
                        if prev_pv is not None:
                            emit_pv(prev_pv)
                        for hh in range(2):
                            h = 2 * pair + hh
                            srow = att.tile([1, CHUNK], F32, tag="srow",
                                            name="srow", bufs=4)
                            rec = att.tile([1, CHUNK], F32, tag="rec",
                                           name="rec", bufs=4)
                            bc = att.tile([64, CHUNK], F32, tag="bc",
                                          name="bc", bufs=4)
                            nc.vector.tensor_copy(srow[:], pv[hh][64:65, :])
                            nc.vector.reciprocal_approx_fast(rec[:], srow[:])
                            nc.gpsimd.partition_broadcast(bc[:], rec[:])
                            dst = attnt1 if h < 2 else attnt2
                            r0 = 64 * (h % 2)
                            nc.vector.tensor_mul(dst[r0:r0 + 64, gq],
                                                 pv[hh][0:64, :], bc[:])
                    m0 = (b * SEQ + q0) // KC
                    pending.extend((m0 + j, nh)
                                   for j in range(CHUNK // KC)
                                   for nh in range(2))
            # flush: scores are done, so the "s" buffers are free — alternate
            # tags for a 3-deep fill/evac pipeline on the tail units
            fl = 0
            while pending:
                if fl % 2 == 0:
                    emit_wo(pending.pop(0))
                else:
                    emit_wo(pending.pop(0), tag="s", bufs=2)
                fl += 1

    nc.compile()
    return nc


def _classify(mask):
    """Classify (qc, kc) tiles. Returns (tile_types, generic)."""
    masked = mask <= -1e8
    zero = mask == 0.0
    tri = np.tril(np.ones((SEQ, SEQ), dtype=bool))  # keep where q >= k
    causal = bool(np.all(zero | masked)) and bool(
        np.array_equal(~masked, tri))
    types = [[None] * NKC for _ in range(NQB)]
    if bool(np.all(zero)):
        for qc in range(NQB):
            for kc in range(NKC):
                types[qc][kc] = 'full'
        return types, False
    if causal:
        for qc in range(NQB):
            q0, q1 = qc * CHUNK, qc * CHUNK + CHUNK - 1
            for kc in range(NKC):
                k0, k1 = kc * KC, kc * KC + KC - 1
                if q0 >= k1:
                    types[qc][kc] = 'full'
                elif q1 < k0:
                    types[qc][kc] = 'skip'
                else:
                    types[qc][kc] = 'diag'
        return types, False
    for qc in range(NQB):
        sub_q = slice(qc * CHUNK, (qc + 1) * CHUNK)
        for kc in range(NKC):
            sub = mask[sub_q, kc * KC:(kc + 1) * KC]
            if np.all(sub == 0.0):
                types[qc][kc] = 'full'
            elif np.all(sub <= -1e8):
                types[qc][kc] = 'skip'
            else:
                types[qc][kc] = 'gen'
    return types, True


def kernel(x, freqs_cos, freqs_sin, mask, wq, wk, wv, wo, cache_k, cache_v,
           start_pos):
    global LAST_RESULT
    from concourse import bass_utils

    x = np.asarray(x, dtype=np.float32)
    freqs_cos = np.asarray(freqs_cos, dtype=np.float32)
    freqs_sin = np.asarray(freqs_sin, dtype=np.float32)
    mask = np.asarray(mask, dtype=np.float32)
    wq = np.asarray(wq, dtype=np.float32)
    wk = np.asarray(wk, dtype=np.float32)
    wv = np.asarray(wv, dtype=np.float32)
    wo = np.asarray(wo, dtype=np.float32)
    assert int(start_pos) == 0, "kernel assumes start_pos == 0"

    tile_types, generic = _classify(mask)
    key = (tuple(tuple(r) for r in tile_types), generic)
    if key not in _CACHE:
        _CACHE[key] = _build(tile_types, generic)
    nc = _CACHE[key]

    import ml_dtypes
    bf16 = ml_dtypes.bfloat16
    # chunk-major packing: xt[ch, p, t*CHUNK+n] = x.T[t*128+p, ch*CHUNK+n]
    # so every projection DMA reads contiguous 16KB per partition
    xT = x.reshape(TOK, DIM).T.astype(bf16)
    xt = np.ascontiguousarray(
        xT.reshape(DIM // KC, KC, TOK // CHUNK, CHUNK)
        .transpose(2, 1, 0, 3).reshape(TOK // CHUNK, KC, -1))
    cos2 = np.concatenate([freqs_cos.T, freqs_cos.T], axis=1)  # [32, 4096]
    sin2 = np.concatenate([freqs_sin.T, freqs_sin.T], axis=1)
    cos_q = np.ascontiguousarray(
        np.tile(cos2, (4, 1)).reshape(KC, TOK // CHUNK, CHUNK)
        .transpose(1, 0, 2)).astype(bf16)
    sin_q = np.ascontiguousarray(
        np.tile(sin2, (4, 1)).reshape(KC, TOK // CHUNK, CHUNK)
        .transpose(1, 0, 2)).astype(bf16)
    maskt = np.ascontiguousarray(mask.T) if generic else None

    def pack_w(w):  # [DIM, M] -> [KC, (DIM//KC)*M] per-partition contiguous
        m = w.shape[1]
        return np.ascontiguousarray(
            w.reshape(DIM // KC, KC, m).transpose(1, 0, 2)
            .reshape(KC, -1)).astype(bf16)

    ev = np.arange(0, HEAD_DIM, 2)
    od = np.arange(1, HEAD_DIM, 2)
    in_maps = []
    for c in range(N_CORES):
        heads = [HPC * c + i for i in range(HPC)]
        qa_cols = np.concatenate([h * HEAD_DIM + ev for h in heads])
        qb_cols = np.concatenate([h * HEAD_DIM + od for h in heads])
        wq_shard = pack_w(np.concatenate([wq[:, qa_cols], wq[:, qb_cols]],
                                         axis=1))
        wkv = pack_w(np.concatenate(
            [wk[:, c * HEAD_DIM + ev], wk[:, c * HEAD_DIM + od],
             wv[:, c * HEAD_DIM:(c + 1) * HEAD_DIM]], axis=1))
        wo_rows = wo[heads[0] * HEAD_DIM:(heads[-1] + 1) * HEAD_DIM, :]
        m = {"xt": xt, "cos_q": cos_q, "sin_q": sin_q,
             "wq": wq_shard, "wkv": wkv,
             "wo1": np.ascontiguousarray(wo_rows[0:128]).astype(bf16),
             "wo2": np.ascontiguousarray(wo_rows[128:256]).astype(bf16)}
        if generic:
            m["maskt"] = maskt
        in_maps.append(m)

    res = bass_utils.run_bass_kernel_spmd(nc, in_maps, list(range(N_CORES)))
    LAST_RESULT = res
    total = np.zeros((TOK // KC, 2, KC, DIM // 2), dtype=np.float32)
    for c in range(N_CORES):
        total += np.asarray(res.results[c]["out"], dtype=np.float32)
    # block-major [m, half, 128, 1024] -> [tok, dim]
    return np.ascontiguousarray(total.transpose(0, 2, 1, 3)).reshape(
        BATCH, SEQ, DIM)



# revision 42
# speedup vs baseline: 1.0855x; 1.0855x over previous
"""Trainium2 Bass kernel for nn_Attention_56736517980223.

Full-input contract: kernel(**inputs) takes the unsharded inputs and returns
the full [2, 2048, 2048] attention output. Internally: tensor-parallel over
heads across 8 NeuronCores (1 KV head + 4 Q heads per core); each core
computes its heads' attention and a partial x@wo contribution; the host sums
the 8 partials.

Device-side dataflow per core (matmuls in bf16 with fp32 PSUM accumulate):
  - xT (host-pretransposed [dim, tok]) streams as the moving operand of the
    QKV projections; weight columns are even/odd-permuted so RoPE acts on
    contiguous 32-partition blocks.
  - scores are computed transposed (S^T[k, q] = kT.T-chunk @ qT) so softmax
    needs no reductions: exp(scale*s) runs on ScalarE straight out of PSUM,
    causal masking is a staircase zero-fill (gpsimd affine_select), and the
    row sums fall out of the PV matmul via a ones-column appended to V.
  - PV output [d+1, q] is normalized with reciprocal + partition_broadcast
    and written into attnT, which feeds the wo matmul as the stationary
    operand.
  - The wo projection is interleaved into the attention loop (one
    [128-token x 1024-col] unit per two kc iterations) so its matmuls fill
    the PE idle left by ScalarE's exp gating; output partials are written
    bf16 and summed on the host in fp32.
"""

import numpy as np

DIM = 2048
N_HEADS = 32
N_KV_HEADS = 8
HEAD_DIM = 64
BATCH = 2
SEQ = 2048
TOK = BATCH * SEQ  # 4096
N_CORES = 8
HPC = N_HEADS // N_KV_HEADS  # 4 q heads per core
CHUNK = 512  # token chunk (projection streaming / q block)
KC = 128     # key chunk (scores partition dim)
NQB = SEQ // CHUNK   # 4 q blocks per batch
NKC = SEQ // KC      # 16 key chunks per batch
SCALE = 1.0 / np.sqrt(HEAD_DIM)

_CACHE = {}
LAST_RESULT = None


def _build(tile_types, generic):
    """Build the SPMD Bass program.

    tile_types[qc][kc] in {'full', 'diag', 'skip', 'gen'} (batch-local,
    shared across batches and heads). 'diag' uses the causal affine_select;
    'gen' adds a DMA'd mask tile (only in generic mode).
    """
    from contextlib import ExitStack
    import concourse.bass as bass
    import concourse.tile as tile
    from concourse import bacc, mybir

    F32 = mybir.dt.float32
    F32R = mybir.dt.float32r
    BF16 = mybir.dt.bfloat16
    U16 = mybir.dt.uint16
    AF = mybir.ActivationFunctionType
    ALU = mybir.AluOpType

    nc = bacc.Bacc("TRN2", target_bir_lowering=False, debug=False,
                   num_devices=N_CORES)

    NCH = TOK // CHUNK  # 8 token chunks
    NKT = DIM // KC     # 16 contraction tiles for projections

    # All streaming inputs are host-packed so every DMA reads per-partition
    # CONTIGUOUS bytes (128 big descriptors instead of 2048 x 1KB gathers);
    # out is block-major per [128-token x 1024-col] wo unit for the same
    # reason, and the host reassembles.
    xt = nc.dram_tensor("xt", [NCH, KC, NKT * CHUNK], BF16,
                        kind="ExternalInput").ap()
    wq = nc.dram_tensor("wq", [KC, NKT * 2 * KC], BF16,
                        kind="ExternalInput").ap()
    wkv = nc.dram_tensor("wkv", [KC, NKT * KC], BF16,
                         kind="ExternalInput").ap()
    wo1 = nc.dram_tensor("wo1", [KC, DIM], BF16, kind="ExternalInput").ap()
    wo2 = nc.dram_tensor("wo2", [KC, DIM], BF16, kind="ExternalInput").ap()
    cos_q = nc.dram_tensor("cos_q", [NCH, KC, CHUNK], BF16,
                           kind="ExternalInput").ap()
    sin_q = nc.dram_tensor("sin_q", [NCH, KC, CHUNK], BF16,
                           kind="ExternalInput").ap()
    if generic:
        maskt = nc.dram_tensor("maskt", [SEQ, SEQ], F32,
                               kind="ExternalInput").ap()
    out = nc.dram_tensor("out", [TOK // KC, 2, KC, DIM // 2], BF16,
                         kind="ExternalOutput").ap()

    with tile.TileContext(nc) as tc, ExitStack() as ctx:
        persist = ctx.enter_context(tc.tile_pool(name="persist", bufs=1))
        qt1 = persist.tile([KC, TOK], BF16)   # heads 0,1 (rows 0:64 / 64:128)
        qt2 = persist.tile([KC, TOK], BF16)   # heads 2,3
        kt = persist.tile([KC, TOK], BF16)    # rows 0:64 = kT, 64:128 = dup
        ident = persist.tile([64, 64], BF16)
        nc.gpsimd.memset(ident[:].bitcast(U16), 0)
        nc.gpsimd.affine_select(
            out=ident[:], in_=ident[:], compare_op=ALU.not_equal,
            fill=1.0, base=0, channel_multiplier=1, pattern=[[-1, 64]])

        vpool = ctx.enter_context(tc.tile_pool(name="vpool", bufs=1))
        vt = vpool.tile([64, TOK], BF16)
        v_all = vpool.tile([KC, (TOK // KC) * 65], BF16)  # 32 [128,65] blocks

        # ---------------- projection + RoPE + V transpose ----------------
        with tc.tile_pool(name="proj", bufs=1) as proj, \
             tc.tile_pool(name="projs", bufs=3) as projs, \
             tc.tile_pool(name="ropet", bufs=2) as ropet, \
             tc.tile_pool(name="pps", bufs=2, space="PSUM") as pps:
            wq_sb = proj.tile([KC, NKT * 2 * KC], BF16)
            wkv_sb = proj.tile([KC, NKT * KC], BF16)
            nc.sync.dma_start(wq_sb[:], wq[:])
            nc.sync.dma_start(wkv_sb[:], wkv[:])

            def emit_transposes(tch):
                for j in range(CHUNK // KC):
                    blk = tch * (CHUNK // KC) + j
                    vp = pps.tile([KC, 64], BF16, tag="vp", name="vp",
                                  bufs=2)
                    nc.tensor.transpose(
                        vp[:], vt[0:64, blk * KC:(blk + 1) * KC], ident[:])
                    nc.scalar.copy(v_all[:, blk * 65:blk * 65 + 64], vp[:])
                    nc.gpsimd.memset(
                        v_all[:, blk * 65 + 64:blk * 65 + 65].bitcast(U16),
                        16256)

            for ch in range(NCH):
                tsl = slice(ch * CHUNK, (ch + 1) * CHUNK)
                # whole x chunk in one contiguous per-partition DMA
                xc = projs.tile([KC, NKT * CHUNK], BF16, tag="xc")
                nc.sync.dma_start(xc[:], xt[ch])
                cos_sb = projs.tile([KC, CHUNK], BF16)
                sin_sb = projs.tile([KC, CHUNK], BF16)
                nc.sync.dma_start(cos_sb[:], cos_q[ch])
                nc.sync.dma_start(sin_sb[:], sin_q[ch])

                a_ps = pps.tile([KC, CHUNK], F32, tag="a_ps")
                b_ps = pps.tile([KC, CHUNK], F32, tag="b_ps")
                kv_ps = pps.tile([KC, CHUNK], F32, tag="kv_ps")
                # group matmuls per accumulator: dense same-bank runs keep
                # the PE stream from cycling PSUM banks every instruction
                for ps_t, woff, wsb in ((a_ps, 0, wq_sb), (b_ps, 128, wq_sb),
                                        (kv_ps, 0, wkv_sb)):
                    stride = 256 if wsb is wq_sb else 128
                    for kti in range(NKT):
                        nc.tensor.matmul(
                            ps_t[:],
                            wsb[:, kti * stride + woff:
                                kti * stride + woff + 128],
                            xc[:, kti * CHUNK:(kti + 1) * CHUNK],
                            start=(kti == 0), stop=(kti == NKT - 1))

                # Q RoPE: A' = A*c - B*s ; B' = A*s + B*c. VectorE does the
                # four PSUM-reading multiplies full-width; GpSimd (idle here,
                # SBUF-only) combines 32-row slices straight into the
                # per-head-contiguous qt1/qt2 rows, keeping the qa/qb scatter
                # off the saturated DMA pipe.
                t1 = ropet.tile([KC, CHUNK], F32, tag="t1")
                t2 = ropet.tile([KC, CHUNK], F32, tag="t2")
                qa = ropet.tile([KC, CHUNK], BF16, tag="qa")
                qb = ropet.tile([KC, CHUNK], BF16, tag="qb")
                nc.vector.tensor_mul(t1[:], a_ps[:], cos_sb[:])
                nc.vector.tensor_mul(t2[:], b_ps[:], sin_sb[:])
                nc.vector.tensor_sub(qa[:], t1[:], t2[:])
                t3 = ropet.tile([KC, CHUNK], F32, tag="t1")
                t4 = ropet.tile([KC, CHUNK], F32, tag="t2")
                nc.vector.tensor_mul(t3[:], a_ps[:], sin_sb[:])
                nc.vector.tensor_mul(t4[:], b_ps[:], cos_sb[:])
                nc.vector.tensor_add(qb[:], t3[:], t4[:])
                for h in range(HPC):
                    dst = qt1 if h < 2 else qt2
                    r0 = 64 * (h % 2)
                    nc.sync.dma_start(dst[r0:r0 + 32, tsl],
                                      qa[32 * h:32 * h + 32, :])
                    nc.sync.dma_start(dst[r0 + 32:r0 + 64, tsl],
                                      qb[32 * h:32 * h + 32, :])

                # K RoPE into kt rows 0:64 (kv_ps rows 0:32=x0, 32:64=x1)
                k1 = ropet.tile([32, CHUNK], F32, tag="k1")
                k2 = ropet.tile([32, CHUNK], F32, tag="k2")
                nc.vector.tensor_mul(k1[:], kv_ps[0:32, :], cos_sb[0:32, :])
                nc.vector.tensor_mul(k2[:], kv_ps[32:64, :], sin_sb[0:32, :])
                nc.vector.tensor_sub(kt[0:32, tsl], k1[:], k2[:])
                k3 = ropet.tile([32, CHUNK], F32, tag="k1")
                k4 = ropet.tile([32, CHUNK], F32, tag="k2")
                nc.vector.tensor_mul(k3[:], kv_ps[0:32, :], sin_sb[0:32, :])
                nc.vector.tensor_mul(k4[:], kv_ps[32:64, :], cos_sb[0:32, :])
                nc.vector.tensor_add(kt[32:64, tsl], k3[:], k4[:])
                # duplicate kT rows for base-64 rhs matmuls
                nc.gpsimd.tensor_copy(kt[64:128, tsl], kt[0:64, tsl])
                # V: evacuate psum rows 64:128 to vt on ScalarE (idle in this
                # phase); the transposes are deferred one chunk so the PE
                # stream never waits on this chunk's evacuation
                nc.scalar.copy(vt[0:64, tsl], kv_ps[64:128, :])
                if ch > 0:
                    emit_transposes(ch - 1)
            emit_transposes(NCH - 1)

        # ---------------- attention ----------------
        attn = ctx.enter_context(tc.tile_pool(name="attn", bufs=1))
        attnt1 = attn.tile([KC, TOK], BF16)
        attnt2 = attn.tile([KC, TOK], BF16)
        wo1_sb = attn.tile([KC, DIM], BF16)
        wo2_sb = attn.tile([KC, DIM], BF16)
        nc.sync.dma_start(wo1_sb[:], wo1[:])
        nc.sync.dma_start(wo2_sb[:], wo2[:])

        with tc.tile_pool(name="att", bufs=2) as att, \
             tc.tile_pool(name="atps", bufs=1, space="PSUM") as atps:
            # wo is interleaved into the attention loop: completed q-blocks
            # queue [128-token x 1024-col] output units that are emitted one
            # per two kc iterations, filling the PE idle left by ScalarE's
            # exp gating. PSUM: s(2x2) + o(2x1) + pv0/pv1(1x1 each) = 8 banks.
            pending = []
            nunit = [0]
            tick = [0]

            def emit_wo(unit):
                m, nh = unit
                msl = slice(m * KC, (m + 1) * KC)
                o_ps = atps.tile([KC, 2 * CHUNK], F32, tag="s",
                                 name="o_ps", bufs=2)
                for j in range(2):
                    nsl = slice(nh * 1024 + j * CHUNK,
                                nh * 1024 + (j + 1) * CHUNK)
                    psl = slice(j * CHUNK, (j + 1) * CHUNK)
                    nc.tensor.matmul(o_ps[:, psl], attnt1[:, msl],
                                     wo1_sb[:, nsl], start=True, stop=False)
                    nc.tensor.matmul(o_ps[:, psl], attnt2[:, msl],
                                     wo2_sb[:, nsl], start=False, stop=True)
                o_sb = att.tile([KC, 2 * CHUNK], BF16, tag="o_sb",
                                name="o_sb", bufs=4)
                nunit[0] += 1
                if nunit[0] % 2 == 0:
                    nc.vector.tensor_copy(o_sb[:], o_ps[:])
                else:
                    nc.scalar.copy(o_sb[:], o_ps[:])
                nc.sync.dma_start(out[m, nh], o_sb[:])

            def maybe_wo():
                # units go through the "s" PSUM tag in PAIRS so the scores
                # double-buffer rotation parity is preserved
                tick[0] += 1
                if tick[0] % 4 == 0 and len(pending) >= 2:
                    emit_wo(pending.pop(0))
                    emit_wo(pending.pop(0))

            for b in range(BATCH):
                for qc in range(NQB):
                    q0 = qc * CHUNK                    # batch-local q offset
                    gq = slice(b * SEQ + q0, b * SEQ + q0 + CHUNK)
                    kcs = [k for k in range(NKC)
                           if tile_types[qc][k] != 'skip']
                    for pair in range(2):
                        qt = qt1 if pair == 0 else qt2
                        pv = [atps.tile([65, CHUNK], F32, tag=f"pv{hh}",
                                        name=f"pv{hh}", bufs=2)
                              for hh in range(2)]

                        def emit_pv(job):
                            ex_t, jw0, jst, jsp, jvblk = job
                            for hh in range(2):
                                csl = slice(hh * CHUNK + jw0,
                                            (hh + 1) * CHUNK)
                                nc.tensor.matmul(
                                    pv[hh][:, jw0:CHUNK],
                                    v_all[:, jvblk * 65:jvblk * 65 + 65],
                                    ex_t[:, csl], start=jst, stop=jsp)
                            maybe_wo()

                        prev_pv = None
                        for i, kci in enumerate(kcs):
                            k0 = kci * KC
                            gk = slice(b * SEQ + k0, b * SEQ + k0 + KC)
                            ty = tile_types[qc][kci]
                            st = (i == 0)
                            sp = (i == len(kcs) - 1)
                            # diag tiles only need q >= k0: shrink to cols
                            # [w0:CHUNK) (earlier cols are fully masked)
                            w0 = max(0, k0 - q0) if ty == 'diag' else 0
                            W = CHUNK - w0
                            gqw = slice(b * SEQ + q0 + w0,
                                        b * SEQ + q0 + CHUNK)
                            s_ps = atps.tile([KC, 2 * CHUNK], F32,
                                             tag="s", name="s_ps", bufs=2)
                            nc.tensor.matmul(
                                s_ps[:, w0:CHUNK], kt[0:64, gk],
                                qt[0:64, gqw], start=True, stop=True)
                            nc.tensor.matmul(
                                s_ps[:, CHUNK + w0:], kt[64:128, gk],
                                qt[64:128, gqw], start=True, stop=True)
                            ex = att.tile([KC, 2 * CHUNK], BF16,
                                          tag="ex", name="ex", bufs=3)
                            if ty == 'gen':
                                mt = att.tile([KC, CHUNK], F32, tag="mt",
                                              name="mt", bufs=4)
                                nc.sync.dma_start(
                                    mt[:], maskt[k0:k0 + KC, q0:q0 + CHUNK])
                                for hh in range(2):
                                    csl = slice(hh * CHUNK, (hh + 1) * CHUNK)
                                    tm = att.tile([KC, CHUNK], F32, tag="tm",
                                                  name="tm", bufs=4)
                                    nc.vector.scalar_tensor_tensor(
                                        tm[:], s_ps[:, csl], SCALE, mt[:],
                                        op0=ALU.mult, op1=ALU.add)
                                    nc.scalar.activation(
                                        ex[:, csl], tm[:], AF.Exp)
                            elif w0 == 0:
                                nc.scalar.activation(
                                    ex[:], s_ps[:], AF.Exp, scale=SCALE)
                            else:
                                sv = s_ps[:].rearrange(
                                    "p (h w) -> p h w", h=2)[:, :, w0:]
                                ev = ex[:].rearrange(
                                    "p (h w) -> p h w", h=2)[:, :, w0:]
                                nc.scalar.activation(ev, sv, AF.Exp,
                                                     scale=SCALE)
                            if ty == 'diag':
                                ev = ex[:].rearrange(
                                    "p (h w) -> p h w", h=2)[:, :, w0:]
                                nc.gpsimd.affine_select(
                                    out=ev, in_=ev,
                                    compare_op=ALU.is_ge, fill=0.0,
                                    base=q0 + w0 - k0,
                                    channel_multiplier=-1,
                                    pattern=[[0, 2], [1, W]])
                            vblk = (b * SEQ + k0) // KC
                            # pv is deferred one iteration: the PE issues the
                            # next tile's scores while ScalarE runs this exp,
                            # so it never stalls on the exp semaphore
                            if prev_pv is not None:
                                emit_pv(prev_pv)
                            prev_pv = (ex, w0, st, sp, vblk)
                        if prev_pv is not None:
                            emit_pv(prev_pv)
                        for hh in range(2):
                            h = 2 * pair + hh
                            srow = att.tile([1, CHUNK], F32, tag="srow",
                                            name="srow", bufs=4)
                            rec = att.tile([1, CHUNK], F32, tag="rec",
                                           name="rec", bufs=4)
                            bc = att.tile([64, CHUNK], F32, tag="bc",
                                          name="bc", bufs=4)
                            nc.vector.tensor_copy(srow[:], pv[hh][64:65, :])
                            nc.vector.reciprocal_approx_fast(rec[:], srow[:])
                            nc.gpsimd.partition_broadcast(bc[:], rec[:])
                            dst = attnt1 if h < 2 else attnt2
                            r0 = 64 * (h % 2)
                            nc.vector.tensor_mul(dst[r0:r0 + 64, gq],
                                                 pv[hh][0:64, :], bc[:])
                    m0 = (b * SEQ + q0) // KC
                    pending.extend((m0 + j, nh)
                                   for j in range(CHUNK // KC)
                                   for nh in range(2))
            while pending:
                emit_wo(pending.pop(0))

    nc.compile()
    return nc


def _classify(mask):
    """Classify (qc, kc) tiles. Returns (tile_types, generic)."""
    masked = mask <= -1e8
    zero = mask == 0.0
    tri = np.tril(np.ones((SEQ, SEQ), dtype=bool))  # keep where q >= k
    causal = bool(np.all(zero | masked)) and bool(
        np.array_equal(~masked, tri))
    types = [[None] * NKC for _ in range(NQB)]
    if bool(np.all(zero)):
        for qc in range(NQB):
            for kc in range(NKC):
                types[qc][kc] = 'full'
        return types, False
    if causal:
        for qc in range(NQB):
            q0, q1 = qc * CHUNK, qc * CHUNK + CHUNK - 1
            for kc in range(NKC):
                k0, k1 = kc * KC, kc * KC + KC - 1
                if q0 >= k1:
                    types[qc][kc] = 'full'
                elif q1 < k0:
                    types[qc][kc] = 'skip'
                else:
                    types[qc][kc] = 'diag'
        return types, False
    for qc in range(NQB):
        sub_q = slice(qc * CHUNK, (qc + 1) * CHUNK)
        for kc in range(NKC):
            sub = mask[sub_q, kc * KC:(kc + 1) * KC]
            if np.all(sub == 0.0):
                types[qc][kc] = 'full'
            elif np.all(sub <= -1e8):
                types[qc][kc] = 'skip'
            else:
                types[qc][kc] = 'gen'
    return types, True


def kernel(x, freqs_cos, freqs_sin, mask, wq, wk, wv, wo, cache_k, cache_v,
           start_pos):
    global LAST_RESULT
    from concourse import bass_utils

    x = np.asarray(x, dtype=np.float32)
    freqs_cos = np.asarray(freqs_cos, dtype=np.float32)
    freqs_sin = np.asarray(freqs_sin, dtype=np.float32)
    mask = np.asarray(mask, dtype=np.float32)
    wq = np.asarray(wq, dtype=np.float32)
    wk = np.asarray(wk, dtype=np.float32)
    wv = np.asarray(wv, dtype=np.float32)
    wo = np.asarray(wo, dtype=np.float32)
    assert int(start_pos) == 0, "kernel assumes start_pos == 0"

    tile_types, generic = _classify(mask)
    key = (tuple(tuple(r) for r in tile_types), generic)
    if key not in _CACHE:
        _CACHE[key] = _build(tile_types, generic)
    nc = _CACHE[key]

    import ml_dtypes
    bf16 = ml_dtypes.bfloat16
    # chunk-major packing: xt[ch, p, t*CHUNK+n] = x.T[t*128+p, ch*CHUNK+n]
    # so every projection DMA reads contiguous 16KB per partition
    xT = x.reshape(TOK, DIM).T.astype(bf16)
    xt = np.ascontiguousarray(
        xT.reshape(DIM // KC, KC, TOK // CHUNK, CHUNK)
        .transpose(2, 1, 0, 3).reshape(TOK // CHUNK, KC, -1))
    cos2 = np.concatenate([freqs_cos.T, freqs_cos.T], axis=1)  # [32, 4096]
    sin2 = np.concatenate([freqs_sin.T, freqs_sin.T], axis=1)
    cos_q = np.ascontiguousarray(
        np.tile(cos2, (4, 1)).reshape(KC, TOK // CHUNK, CHUNK)
        .transpose(1, 0, 2)).astype(bf16)
    sin_q = np.ascontiguousarray(
        np.tile(sin2, (4, 1)).reshape(KC, TOK // CHUNK, CHUNK)
        .transpose(1, 0, 2)).astype(bf16)
    maskt = np.ascontiguousarray(mask.T) if generic else None

    def pack_w(w):  # [DIM, M] -> [KC, (DIM//KC)*M] per-partition contiguous
        m = w.shape[1]
        return np.ascontiguousarray(
            w.reshape(DIM // KC, KC, m).transpose(1, 0, 2)
            .reshape(KC, -1)).astype(bf16)

    ev = np.arange(0, HEAD_DIM, 2)
    od = np.arange(1, HEAD_DIM, 2)
    in_maps = []
    for c in range(N_CORES):
        heads = [HPC * c + i for i in range(HPC)]
        qa_cols = np.concatenate([h * HEAD_DIM + ev for h in heads])
        qb_cols = np.concatenate([h * HEAD_DIM + od for h in heads])
        wq_shard = pack_w(np.concatenate([wq[:, qa_cols], wq[:, qb_cols]],
                                         axis=1))
        wkv = pack_w(np.concatenate(
            [wk[:, c * HEAD_DIM + ev], wk[:, c * HEAD_DIM + od],
             wv[:, c * HEAD_DIM:(c + 1) * HEAD_DIM]], axis=1))
        wo_rows = wo[heads[0] * HEAD_DIM:(heads[-1] + 1) * HEAD_DIM, :]
        m = {"xt": xt, "cos_q": cos_q, "sin_q": sin_q,
             "wq": wq_shard, "wkv": wkv,
             "wo1": np.ascontiguousarray(wo_rows[0:128]).astype(bf16),
             "wo2": np.ascontiguousarray(wo_rows[128:256]).astype(bf16)}
        if generic:
            m["maskt"] = maskt
        in_maps.append(m)

    res = bass_utils.run_bass_kernel_spmd(nc, in_maps, list(range(N_CORES)))
    LAST_RESULT = res
    total = np.zeros((TOK // KC, 2, KC, DIM // 2), dtype=np.float32)
    for c in range(N_CORES):
        total += np.asarray(res.results[c]["out"], dtype=np.float32)
    # block-major [m, half, 128, 1024] -> [tok, dim]
    return np.ascontiguousarray(total.transpose(0, 2, 1, 3)).reshape(
        BATCH, SEQ, DIM)



# revision 46
# speedup vs baseline: 1.0918x; 1.0058x over previous
"""Trainium2 Bass kernel for nn_Attention_56736517980223.

Full-input contract: kernel(**inputs) takes the unsharded inputs and returns
the full [2, 2048, 2048] attention output. Internally: tensor-parallel over
heads across 8 NeuronCores (1 KV head + 4 Q heads per core); each core
computes its heads' attention and a partial x@wo contribution; the host sums
the 8 partials.

Device-side dataflow per core (matmuls in bf16 with fp32 PSUM accumulate):
  - xT (host-pretransposed [dim, tok]) streams as the moving operand of the
    QKV projections; weight columns are even/odd-permuted so RoPE acts on
    contiguous 32-partition blocks.
  - scores are computed transposed (S^T[k, q] = kT.T-chunk @ qT) so softmax
    needs no reductions: exp(scale*s) runs on ScalarE straight out of PSUM,
    causal masking is a staircase zero-fill (gpsimd affine_select), and the
    row sums fall out of the PV matmul via a ones-column appended to V.
  - PV output [d+1, q] is normalized with reciprocal + partition_broadcast
    and written into attnT, which feeds the wo matmul as the stationary
    operand.
  - The wo projection is interleaved into the attention loop (pairs of
    [128-token x 1024-col] units every four kc iterations, sharing the "s"
    PSUM tag with parity preserved) so its matmuls fill the PE idle left by
    ScalarE's exp gating; output partials are written bf16 to block-major
    DRAM and summed/reassembled on the host in fp32.
"""

import numpy as np

DIM = 2048
N_HEADS = 32
N_KV_HEADS = 8
HEAD_DIM = 64
BATCH = 2
SEQ = 2048
TOK = BATCH * SEQ  # 4096
N_CORES = 8
HPC = N_HEADS // N_KV_HEADS  # 4 q heads per core
CHUNK = 512  # token chunk (projection streaming / q block)
KC = 128     # key chunk (scores partition dim)
NQB = SEQ // CHUNK   # 4 q blocks per batch
NKC = SEQ // KC      # 16 key chunks per batch
SCALE = 1.0 / np.sqrt(HEAD_DIM)

_CACHE = {}
LAST_RESULT = None


def _build(tile_types, generic):
    """Build the SPMD Bass program.

    tile_types[qc][kc] in {'full', 'diag', 'skip', 'gen'} (batch-local,
    shared across batches and heads). 'diag' uses the causal affine_select;
    'gen' adds a DMA'd mask tile (only in generic mode).
    """
    from contextlib import ExitStack
    import concourse.bass as bass
    import concourse.tile as tile
    from concourse import bacc, mybir

    F32 = mybir.dt.float32
    F32R = mybir.dt.float32r
    BF16 = mybir.dt.bfloat16
    U16 = mybir.dt.uint16
    AF = mybir.ActivationFunctionType
    ALU = mybir.AluOpType

    nc = bacc.Bacc("TRN2", target_bir_lowering=False, debug=False,
                   num_devices=N_CORES)

    NCH = TOK // CHUNK  # 8 token chunks
    NKT = DIM // KC     # 16 contraction tiles for projections

    # All streaming inputs are host-packed so every DMA reads per-partition
    # CONTIGUOUS bytes (128 big descriptors instead of 2048 x 1KB gathers);
    # out is block-major per [128-token x 1024-col] wo unit for the same
    # reason, and the host reassembles.
    xt = nc.dram_tensor("xt", [NCH, KC, NKT * CHUNK], BF16,
                        kind="ExternalInput").ap()
    wq = nc.dram_tensor("wq", [KC, NKT * 2 * KC], BF16,
                        kind="ExternalInput").ap()
    wkv = nc.dram_tensor("wkv", [KC, NKT * KC], BF16,
                         kind="ExternalInput").ap()
    wo1 = nc.dram_tensor("wo1", [KC, DIM], BF16, kind="ExternalInput").ap()
    wo2 = nc.dram_tensor("wo2", [KC, DIM], BF16, kind="ExternalInput").ap()
    cos_q = nc.dram_tensor("cos_q", [NCH, KC, CHUNK], BF16,
                           kind="ExternalInput").ap()
    sin_q = nc.dram_tensor("sin_q", [NCH, KC, CHUNK], BF16,
                           kind="ExternalInput").ap()
    if generic:
        maskt = nc.dram_tensor("maskt", [SEQ, SEQ], F32,
                               kind="ExternalInput").ap()
    out = nc.dram_tensor("out", [TOK // KC, 2, KC, DIM // 2], BF16,
                         kind="ExternalOutput").ap()

    with tile.TileContext(nc) as tc, ExitStack() as ctx:
        persist = ctx.enter_context(tc.tile_pool(name="persist", bufs=1))
        qt1 = persist.tile([KC, TOK], BF16)   # heads 0,1 (rows 0:64 / 64:128)
        qt2 = persist.tile([KC, TOK], BF16)   # heads 2,3
        kt = persist.tile([KC, TOK], BF16)    # rows 0:64 = kT, 64:128 = dup
        ident = persist.tile([64, 64], BF16)
        nc.gpsimd.memset(ident[:].bitcast(U16), 0)
        nc.gpsimd.affine_select(
            out=ident[:], in_=ident[:], compare_op=ALU.not_equal,
            fill=1.0, base=0, channel_multiplier=1, pattern=[[-1, 64]])

        vpool = ctx.enter_context(tc.tile_pool(name="vpool", bufs=1))
        vt = vpool.tile([64, TOK], BF16)
        v_all = vpool.tile([KC, (TOK // KC) * 65], BF16)  # 32 [128,65] blocks

        # ---------------- projection + RoPE + V transpose ----------------
        with tc.tile_pool(name="proj", bufs=1) as proj, \
             tc.tile_pool(name="projs", bufs=3) as projs, \
             tc.tile_pool(name="ropet", bufs=2) as ropet, \
             tc.tile_pool(name="pps", bufs=2, space="PSUM") as pps:
            wq_sb = proj.tile([KC, NKT * 2 * KC], BF16)
            wkv_sb = proj.tile([KC, NKT * KC], BF16)
            nc.sync.dma_start(wq_sb[:], wq[:])
            nc.sync.dma_start(wkv_sb[:], wkv[:])

            def emit_transposes(tch):
                for j in range(CHUNK // KC):
                    blk = tch * (CHUNK // KC) + j
                    vp = pps.tile([KC, 64], BF16, tag="vp", name="vp",
                                  bufs=2)
                    nc.tensor.transpose(
                        vp[:], vt[0:64, blk * KC:(blk + 1) * KC], ident[:])
                    nc.scalar.copy(v_all[:, blk * 65:blk * 65 + 64], vp[:])
                    nc.gpsimd.memset(
                        v_all[:, blk * 65 + 64:blk * 65 + 65].bitcast(U16),
                        16256)

            for ch in range(NCH):
                tsl = slice(ch * CHUNK, (ch + 1) * CHUNK)
                # whole x chunk in one contiguous per-partition DMA
                xc = projs.tile([KC, NKT * CHUNK], BF16, tag="xc")
                nc.sync.dma_start(xc[:], xt[ch])
                cos_sb = projs.tile([KC, CHUNK], BF16)
                sin_sb = projs.tile([KC, CHUNK], BF16)
                nc.sync.dma_start(cos_sb[:], cos_q[ch])
                nc.sync.dma_start(sin_sb[:], sin_q[ch])

                a_ps = pps.tile([KC, CHUNK], F32, tag="a_ps")
                b_ps = pps.tile([KC, CHUNK], F32, tag="b_ps")
                kv_ps = pps.tile([KC, CHUNK], F32, tag="kv_ps")
                # group matmuls per accumulator: dense same-bank runs keep
                # the PE stream from cycling PSUM banks every instruction
                for ps_t, woff, wsb in ((a_ps, 0, wq_sb), (b_ps, 128, wq_sb),
                                        (kv_ps, 0, wkv_sb)):
                    stride = 256 if wsb is wq_sb else 128
                    for kti in range(NKT):
                        nc.tensor.matmul(
                            ps_t[:],
                            wsb[:, kti * stride + woff:
                                kti * stride + woff + 128],
                            xc[:, kti * CHUNK:(kti + 1) * CHUNK],
                            start=(kti == 0), stop=(kti == NKT - 1))

                # Q RoPE: A' = A*c - B*s ; B' = A*s + B*c. VectorE does the
                # four PSUM-reading multiplies full-width; GpSimd (idle here,
                # SBUF-only) combines 32-row slices straight into the
                # per-head-contiguous qt1/qt2 rows, keeping the qa/qb scatter
                # off the saturated DMA pipe.
                t1 = ropet.tile([KC, CHUNK], F32, tag="t1")
                t2 = ropet.tile([KC, CHUNK], F32, tag="t2")
                qa = ropet.tile([KC, CHUNK], BF16, tag="qa")
                qb = ropet.tile([KC, CHUNK], BF16, tag="qb")
                nc.vector.tensor_mul(t1[:], a_ps[:], cos_sb[:])
                nc.vector.tensor_mul(t2[:], b_ps[:], sin_sb[:])
                nc.vector.tensor_sub(qa[:], t1[:], t2[:])
                t3 = ropet.tile([KC, CHUNK], F32, tag="t1")
                t4 = ropet.tile([KC, CHUNK], F32, tag="t2")
                nc.vector.tensor_mul(t3[:], a_ps[:], sin_sb[:])
                nc.vector.tensor_mul(t4[:], b_ps[:], cos_sb[:])
                nc.vector.tensor_add(qb[:], t3[:], t4[:])
                for h in range(HPC):
                    dst = qt1 if h < 2 else qt2
                    r0 = 64 * (h % 2)
                    nc.sync.dma_start(dst[r0:r0 + 32, tsl],
                                      qa[32 * h:32 * h + 32, :])
                    nc.sync.dma_start(dst[r0 + 32:r0 + 64, tsl],
                                      qb[32 * h:32 * h + 32, :])

                # K RoPE into kt rows 0:64 (kv_ps rows 0:32=x0, 32:64=x1)
                k1 = ropet.tile([32, CHUNK], F32, tag="k1")
                k2 = ropet.tile([32, CHUNK], F32, tag="k2")
                nc.vector.tensor_mul(k1[:], kv_ps[0:32, :], cos_sb[0:32, :])
                nc.vector.tensor_mul(k2[:], kv_ps[32:64, :], sin_sb[0:32, :])
                nc.vector.tensor_sub(kt[0:32, tsl], k1[:], k2[:])
                k3 = ropet.tile([32, CHUNK], F32, tag="k1")
                k4 = ropet.tile([32, CHUNK], F32, tag="k2")
                nc.vector.tensor_mul(k3[:], kv_ps[0:32, :], sin_sb[0:32, :])
                nc.vector.tensor_mul(k4[:], kv_ps[32:64, :], cos_sb[0:32, :])
                nc.vector.tensor_add(kt[32:64, tsl], k3[:], k4[:])
                # duplicate kT rows for base-64 rhs matmuls
                nc.gpsimd.tensor_copy(kt[64:128, tsl], kt[0:64, tsl])
                # V: evacuate psum rows 64:128 to vt on ScalarE (idle in this
                # phase); the transposes are deferred one chunk so the PE
                # stream never waits on this chunk's evacuation
                nc.scalar.copy(vt[0:64, tsl], kv_ps[64:128, :])
                if ch > 0:
                    emit_transposes(ch - 1)
            emit_transposes(NCH - 1)

        # ---------------- attention ----------------
        attn = ctx.enter_context(tc.tile_pool(name="attn", bufs=1))
        attnt1 = attn.tile([KC, TOK], BF16)
        attnt2 = attn.tile([KC, TOK], BF16)
        wo1_sb = attn.tile([KC, DIM], BF16)
        wo2_sb = attn.tile([KC, DIM], BF16)
        nc.sync.dma_start(wo1_sb[:], wo1[:])
        nc.sync.dma_start(wo2_sb[:], wo2[:])

        with tc.tile_pool(name="att", bufs=2) as att, \
             tc.tile_pool(name="atps", bufs=1, space="PSUM") as atps:
            # wo is interleaved into the attention loop: completed q-blocks
            # queue [128-token x 1024-col] output units that are emitted one
            # per two kc iterations, filling the PE idle left by ScalarE's
            # exp gating. PSUM: s(2x2) + o(2x1) + pv0/pv1(1x1 each) = 8 banks.
            pending = []
            nunit = [0]
            tick = [0]

            def emit_wo(unit, flush=False):
                m, nh = unit
                msl = slice(m * KC, (m + 1) * KC)
                o_ps = atps.tile([KC, 2 * CHUNK], F32, tag="s",
                                 name="o_ps", bufs=2)
                for j in range(2):
                    nsl = slice(nh * 1024 + j * CHUNK,
                                nh * 1024 + (j + 1) * CHUNK)
                    psl = slice(j * CHUNK, (j + 1) * CHUNK)
                    nc.tensor.matmul(o_ps[:, psl], attnt1[:, msl],
                                     wo1_sb[:, nsl], start=True, stop=False)
                    nc.tensor.matmul(o_ps[:, psl], attnt2[:, msl],
                                     wo2_sb[:, nsl], start=False, stop=True)
                o_sb = att.tile([KC, 2 * CHUNK], BF16, tag="o_sb",
                                name="o_sb", bufs=4)
                nunit[0] += 1
                # mid-loop evacs stay off ScalarE (it gates the phase with
                # exp); during the flush ScalarE is idle, so alternate there
                if flush and nunit[0] % 2 == 0:
                    nc.scalar.copy(o_sb[:], o_ps[:])
                else:
                    nc.vector.tensor_copy(o_sb[:], o_ps[:])
                nc.sync.dma_start(out[m, nh], o_sb[:])

            def maybe_wo():
                # units go through the "s" PSUM tag in PAIRS so the scores
                # double-buffer rotation parity is preserved
                tick[0] += 1
                if tick[0] % 4 == 0 and len(pending) >= 2:
                    emit_wo(pending.pop(0))
                    emit_wo(pending.pop(0))

            for b in range(BATCH):
                for qc in range(NQB):
                    q0 = qc * CHUNK                    # batch-local q offset
                    gq = slice(b * SEQ + q0, b * SEQ + q0 + CHUNK)
                    kcs = [k for k in range(NKC)
                           if tile_types[qc][k] != 'skip']
                    for pair in range(2):
                        qt = qt1 if pair == 0 else qt2
                        pv = [atps.tile([65, CHUNK], F32, tag=f"pv{hh}",
                                        name=f"pv{hh}", bufs=2)
                              for hh in range(2)]

                        def emit_pv(job):
                            ex_t, jw0, jst, jsp, jvblk = job
                            for hh in range(2):
                                csl = slice(hh * CHUNK + jw0,
                                            (hh + 1) * CHUNK)
                                nc.tensor.matmul(
                                    pv[hh][:, jw0:CHUNK],
                                    v_all[:, jvblk * 65:jvblk * 65 + 65],
                                    ex_t[:, csl], start=jst, stop=jsp)
                            maybe_wo()

                        prev_pv = None
                        for i, kci in enumerate(kcs):
                            k0 = kci * KC
                            gk = slice(b * SEQ + k0, b * SEQ + k0 + KC)
                            ty = tile_types[qc][kci]
                            st = (i == 0)
                            sp = (i == len(kcs) - 1)
                            # diag tiles only need q >= k0: shrink to cols
                            # [w0:CHUNK) (earlier cols are fully masked)
                            w0 = max(0, k0 - q0) if ty == 'diag' else 0
                            W = CHUNK - w0
                            gqw = slice(b * SEQ + q0 + w0,
                                        b * SEQ + q0 + CHUNK)
                            s_ps = atps.tile([KC, 2 * CHUNK], F32,
                                             tag="s", name="s_ps", bufs=2)
                            nc.tensor.matmul(
                                s_ps[:, w0:CHUNK], kt[0:64, gk],
                                qt[0:64, gqw], start=True, stop=True)
                            nc.tensor.matmul(
                                s_ps[:, CHUNK + w0:], kt[64:128, gk],
                                qt[64:128, gqw], start=True, stop=True)
                            ex = att.tile([KC, 2 * CHUNK], BF16,
                                          tag="ex", name="ex", bufs=3)
                            if ty == 'gen':
                                mt = att.tile([KC, CHUNK], F32, tag="mt",
                                              name="mt", bufs=4)
                                nc.sync.dma_start(
                                    mt[:], maskt[k0:k0 + KC, q0:q0 + CHUNK])
                                for hh in range(2):
                                    csl = slice(hh * CHUNK, (hh + 1) * CHUNK)
                                    tm = att.tile([KC, CHUNK], F32, tag="tm",
                                                  name="tm", bufs=4)
                                    nc.vector.scalar_tensor_tensor(
                                        tm[:], s_ps[:, csl], SCALE, mt[:],
                                        op0=ALU.mult, op1=ALU.add)
                                    nc.scalar.activation(
                                        ex[:, csl], tm[:], AF.Exp)
                            elif w0 == 0:
                                nc.scalar.activation(
                                    ex[:], s_ps[:], AF.Exp, scale=SCALE)
                            else:
                                sv = s_ps[:].rearrange(
                                    "p (h w) -> p h w", h=2)[:, :, w0:]
                                ev = ex[:].rearrange(
                                    "p (h w) -> p h w", h=2)[:, :, w0:]
                                nc.scalar.activation(ev, sv, AF.Exp,
                                                     scale=SCALE)
                            if ty == 'diag':
                                ev = ex[:].rearrange(
                                    "p (h w) -> p h w", h=2)[:, :, w0:]
                                nc.gpsimd.affine_select(
                                    out=ev, in_=ev,
                                    compare_op=ALU.is_ge, fill=0.0,
                                    base=q0 + w0 - k0,
                                    channel_multiplier=-1,
                                    pattern=[[0, 2], [1, W]])
                            vblk = (b * SEQ + k0) // KC
                            # pv is deferred one iteration: the PE issues the
                            # next tile's scores while ScalarE runs this exp,
                            # so it never stalls on the exp semaphore
                            if prev_pv is not None:
                                emit_pv(prev_pv)
                            prev_pv = (ex, w0, st, sp, vblk)
                        if prev_pv is not None:
                            emit_pv(prev_pv)
                        for hh in range(2):
                            h = 2 * pair + hh
                            srow = att.tile([1, CHUNK], F32, tag="srow",
                                            name="srow", bufs=4)
                            rec = att.tile([1, CHUNK], F32, tag="rec",
                                           name="rec", bufs=4)
                            bc = att.tile([64, CHUNK], F32, tag="bc",
                                          name="bc", bufs=4)
                            nc.vector.tensor_copy(srow[:], pv[hh][64:65, :])
                            nc.vector.reciprocal_approx_fast(rec[:], srow[:])
                            nc.gpsimd.partition_broadcast(bc[:], rec[:])
                            dst = attnt1 if h < 2 else attnt2
                            r0 = 64 * (h % 2)
                            nc.vector.tensor_mul(dst[r0:r0 + 64, gq],
                                                 pv[hh][0:64, :], bc[:])
                    m0 = (b * SEQ + q0) // KC
                    pending.extend((m0 + j, nh)
                                   for j in range(CHUNK // KC)
                                   for nh in range(2))
            while pending:
                emit_wo(pending.pop(0), flush=True)

    nc.compile()
    return nc


def _classify(mask):
    """Classify (qc, kc) tiles. Returns (tile_types, generic)."""
    masked = mask <= -1e8
    zero = mask == 0.0
    tri = np.tril(np.ones((SEQ, SEQ), dtype=bool))  # keep where q >= k
    causal = bool(np.all(zero | masked)) and bool(
        np.array_equal(~masked, tri))
    types = [[None] * NKC for _ in range(NQB)]
    if bool(np.all(zero)):
        for qc in range(NQB):
            for kc in range(NKC):
                types[qc][kc] = 'full'
        return types, False
    if causal:
        for qc in range(NQB):
            q0, q1 = qc * CHUNK, qc * CHUNK + CHUNK - 1
            for kc in range(NKC):
                k0, k1 = kc * KC, kc * KC + KC - 1
                if q0 >= k1:
                    types[qc][kc] = 'full'
                elif q1 < k0:
                    types[qc][kc] = 'skip'
                else:
                    types[qc][kc] = 'diag'
        return types, False
    for qc in range(NQB):
        sub_q = slice(qc * CHUNK, (qc + 1) * CHUNK)
        for kc in range(NKC):
            sub = mask[sub_q, kc * KC:(kc + 1) * KC]
            if np.all(sub == 0.0):
                types[qc][kc] = 'full'
            elif np.all(sub <= -1e8):
                types[qc][kc] = 'skip'
            else:
                types[qc][kc] = 'gen'
    return types, True


def kernel(x, freqs_cos, freqs_sin, mask, wq, wk, wv, wo, cache_k, cache_v,
           start_pos):
    global LAST_RESULT
    from concourse import bass_utils

    x = np.asarray(x, dtype=np.float32)
    freqs_cos = np.asarray(freqs_cos, dtype=np.float32)
    freqs_sin = np.asarray(freqs_sin, dtype=np.float32)
    mask = np.asarray(mask, dtype=np.float32)
    wq = np.asarray(wq, dtype=np.float32)
    wk = np.asarray(wk, dtype=np.float32)
    wv = np.asarray(wv, dtype=np.float32)
    wo = np.asarray(wo, dtype=np.float32)
    assert int(start_pos) == 0, "kernel assumes start_pos == 0"

    tile_types, generic = _classify(mask)
    key = (tuple(tuple(r) for r in tile_types), generic)
    if key not in _CACHE:
        _CACHE[key] = _build(tile_types, generic)
    nc = _CACHE[key]

    import ml_dtypes
    bf16 = ml_dtypes.bfloat16
    # chunk-major packing: xt[ch, p, t*CHUNK+n] = x.T[t*128+p, ch*CHUNK+n]
    # so every projection DMA reads contiguous 16KB per partition
    xT = x.reshape(TOK, DIM).T.astype(bf16)
    xt = np.ascontiguousarray(
        xT.reshape(DIM // KC, KC, TOK // CHUNK, CHUNK)
        .transpose(2, 1, 0, 3).reshape(TOK // CHUNK, KC, -1))
    cos2 = np.concatenate([freqs_cos.T, freqs_cos.T], axis=1)  # [32, 4096]
    sin2 = np.concatenate([freqs_sin.T, freqs_sin.T], axis=1)
    cos_q = np.ascontiguousarray(
        np.tile(cos2, (4, 1)).reshape(KC, TOK // CHUNK, CHUNK)
        .transpose(1, 0, 2)).astype(bf16)
    sin_q = np.ascontiguousarray(
        np.tile(sin2, (4, 1)).reshape(KC, TOK // CHUNK, CHUNK)
        .transpose(1, 0, 2)).astype(bf16)
    maskt = np.ascontiguousarray(mask.T) if generic else None

    def pack_w(w):  # [DIM, M] -> [KC, (DIM//KC)*M] per-partition contiguous
        m = w.shape[1]
        return np.ascontiguousarray(
            w.reshape(DIM // KC, KC, m).transpose(1, 0, 2)
            .reshape(KC, -1)).astype(bf16)

    ev = np.arange(0, HEAD_DIM, 2)
    od = np.arange(1, HEAD_DIM, 2)
    in_maps = []
    for c in range(N_CORES):
        heads = [HPC * c + i for i in range(HPC)]
        qa_cols = np.concatenate([h * HEAD_DIM + ev for h in heads])
        qb_cols = np.concatenate([h * HEAD_DIM + od for h in heads])
        wq_shard = pack_w(np.concatenate([wq[:, qa_cols], wq[:, qb_cols]],
                                         axis=1))
        wkv = pack_w(np.concatenate(
            [wk[:, c * HEAD_DIM + ev], wk[:, c * HEAD_DIM + od],
             wv[:, c * HEAD_DIM:(c + 1) * HEAD_DIM]], axis=1))
        wo_rows = wo[heads[0] * HEAD_DIM:(heads[-1] + 1) * HEAD_DIM, :]
        m = {"xt": xt, "cos_q": cos_q, "sin_q": sin_q,
             "wq": wq_shard, "wkv": wkv,
             "wo1": np.ascontiguousarray(wo_rows[0:128]).astype(bf16),
             "wo2": np.ascontiguousarray(wo_rows[128:256]).astype(bf16)}
        if generic:
            m["maskt"] = maskt
        in_maps.append(m)

    res = bass_utils.run_bass_kernel_spmd(nc, in_maps, list(range(N_CORES)))
    LAST_RESULT = res
    total = np.zeros((TOK // KC, 2, KC, DIM // 2), dtype=np.float32)
    for c in range(N_CORES):
        total += np.asarray(res.results[c]["out"], dtype=np.float32)
    # block-major [m, half, 128, 1024] -> [tok, dim]
    return np.ascontiguousarray(total.transpose(0, 2, 1, 3)).reshape(
        BATCH, SEQ, DIM)

